# revision 1
# baseline (speedup 1.0000x reference)
"""Multi-head self-attention on 8 Trainium2 NeuronCores.

Problem: x[2, 4096, 768], Wq/Wk/Wv[768, 512], Wout[512, 768], b_out[768]
  q,k,v = heads(x@W*); S = qk^T/8; P = softmax(S); out = (P v) @ Wout + b_out
Sharding: 16 (batch, head) pairs -> 8 cores, 2 heads each (core i: batch i//4,
  heads 2*(i%4), 2*(i%4)+1). Each core holds its weight slices and computes a
  partial y^T[768, 4096]; host sums the 4 per-batch partials and adds b_out.

Device dataflow (all matmuls bf16, fp32 PSUM accumulation):
  x^T (transposed+cast on host)  ->  q^T,k^T [128, 4096]  (W stationary)
  v natural [4096, 128] via x^T-stationary matmuls, stored with a ones
    column per head (v_ext[j, h, 0:65], col 64 = 1.0)
  S^T[j,i] both heads per j-tile via row-tiled (tile_position (0,0)/(64,0))
    K=64 matmul pairs into [128, 1536] PSUM chunks (3 banks, double-buffered)
  P^T = exp(S^T/8) on ScalarE in 1536-wide ops (no max subtraction; scores
    are O(5) so fp32 exp is safe) -- ScalarE is the bottleneck engine
  AV with P^T STATIONARY (lhsT): per (head, i-sub 128) chain over 32 j-tiles,
    rhs = v_ext[j, h, 0:65] (N=65) -> o^T[i, 0:64] and Z[i] in col 64 of the
    same PSUM accumulation, so the softmax denominator is free (this removes
    the DVE zacc accumulation and the z-broadcast matmuls of the previous
    version, and halves the AV charge on PE: 65 output rows vs 2x512).
  normalize: rz = 1/Z (DVE reciprocal), o^T scaled per-partition into bf16
    oTn (DVE tensor_scalar_mul); PE-transpose oTn -> o[d, i] (identity rhs)
  out-proj: single K=128 matmul per 128-row slice of Wout; tail of chunk i
    is emitted after chunk i+1's S/exp so it never gates ScalarE.
  Bulk PSUM->SBUF copies (q^T/k^T, v, y) run on the otherwise idle GpSimd
    (Pool) engine.
"""
import os
import numpy as np
import ml_dtypes

ABLATE = set(os.environ.get("KABLATE", "").split(","))
KITER = int(os.environ.get("KITER", "1"))

import concourse.bass as bass
import concourse.mybir as mybir
import concourse.tile as tile
from concourse import bacc
from concourse.bass_utils import run_bass_kernel_spmd

BF16 = ml_dtypes.bfloat16
F32 = mybir.dt.float32
BF = mybir.dt.bfloat16

B, N, QDIM = 2, 4096, 768
H, D = 8, 64
KT = QDIM // 128          # 6 contraction tiles
NCH = N // 512            # 8 i-chunks
NJT = N // 128            # 32 j-tiles
SCALE = D ** -0.5         # 1/8

# DVE/Pool polynomial exp (for the ~1/8 of softmax tiles offloaded from the
# busy ScalarE): exp(s/8) = p(z)^8 with z = s/64, p = degree-5 minimax-ish
# factorization 1/120*(z-r0)*((z+b1)^2+g1)*((z+b2)^2+g2) evaluated in fp16
# on DVE via tensor_scalar (4x) + tensor_tensor (2x) ops, constants scaled
# by ca = (1/120)^(1/5) distributed across the three factors.
EK1 = 0.0059976867   # ca/64
EK2 = 0.8370303      # -ca*r0
ES1 = -0.92908045    # ca*(b1+r0)
EG1 = 1.44196267     # g1*ca^2
ES2 = -0.20386542    # ca*(b2+r0)
EG2 = 0.42278558     # g2*ca^2


def _body(ctx, tc):
    nc = tc.nc

    xT = nc.dram_tensor("xT", [QDIM, N], BF, kind="ExternalInput").ap()
    wq = nc.dram_tensor("wq", [QDIM, 128], BF, kind="ExternalInput").ap()
    wk = nc.dram_tensor("wk", [QDIM, 128], BF, kind="ExternalInput").ap()
    wv = nc.dram_tensor("wv", [QDIM, 128], BF, kind="ExternalInput").ap()
    wout = nc.dram_tensor("wout", [128, QDIM], BF, kind="ExternalInput").ap()
    ident = nc.dram_tensor("ident", [128, 128], BF, kind="ExternalInput").ap()
    yT = nc.dram_tensor("yT", [QDIM, N], F32, kind="ExternalOutput").ap()

    xT_r = xT.rearrange("(k p) n -> p k n", p=128)
    wq_r = wq.rearrange("(k p) m -> p k m", p=128)
    wk_r = wk.rearrange("(k p) m -> p k m", p=128)
    wv_r = wv.rearrange("(k p) m -> p k m", p=128)
    wout_r = wout.rearrange("p (k f) -> p k f", f=128)
    yT_r = yT.rearrange("(m p) n -> m p n", p=128)

    # ---- static SBUF ----
    singles = ctx.enter_context(tc.tile_pool(name="singles", bufs=1))
    xT_sb = singles.tile([128, KT, N], BF, name="xT_sb", tag="xT_sb")
    wq_sb = singles.tile([128, KT, 128], BF, name="wq_sb", tag="wq_sb")
    wk_sb = singles.tile([128, KT, 128], BF, name="wk_sb", tag="wk_sb")
    wv_sb = singles.tile([128, KT, 128], BF, name="wv_sb", tag="wv_sb")
    wout_sb = singles.tile([128, KT, 128], BF, name="wout_sb", tag="wout_sb")
    id_sb = singles.tile([128, 128], BF, name="id_sb", tag="id_sb")
    qT_sb = singles.tile([128, N], BF, name="qT_sb", tag="qT_sb")
    kT_sb = singles.tile([128, N], BF, name="kT_sb", tag="kT_sb")
    # v per j-tile and head, with a trailing ones column: [j, jt, h, 0:64]=v,
    # [j, jt, h, 64]=1.0 (the AV rhs [j, 65] then accumulates Z in out col 64).
    # Double-buffered across KITER iterations so iteration n+1's v projection
    # does not serialize behind iteration n's AV reads.
    v_sb = singles.tile([128, 2, NJT, 2, 65], BF, name="v_sb", tag="v_sb")

    for k in range(KT):
        for q4 in range(4):
            qs = bass.ts(q4, N // 4)
            nc.sync.dma_start(out=xT_sb[:, k, qs], in_=xT_r[:, k, qs])
    nc.sync.dma_start(out=wq_sb, in_=wq_r)
    nc.sync.dma_start(out=wk_sb, in_=wk_r)
    nc.sync.dma_start(out=wv_sb, in_=wv_r)
    nc.sync.dma_start(out=wout_sb, in_=wout_r)
    nc.sync.dma_start(out=id_sb, in_=ident)
    nc.vector.memset(v_sb[:, :, :, :, 64], 1.0)

    psA = ctx.enter_context(tc.tile_pool(name="psA", bufs=3, space="PSUM"))
    psB = ctx.enter_context(tc.tile_pool(name="psB", bufs=2, space="PSUM"))
    ptp = ctx.enter_context(tc.tile_pool(name="ptp", bufs=6))
    sm = ctx.enter_context(tc.tile_pool(name="sm", bufs=2))
    yp = ctx.enter_context(tc.tile_pool(name="yp", bufs=2))
    ep = ctx.enter_context(tc.tile_pool(name="ep", bufs=1))

    from collections import deque
    fillers = deque()
    pending = []
    for _it in range(KITER):
        _compute(nc, psA, psB, ptp, sm, yp, ep,
                 xT_sb, wq_sb, wk_sb, wv_sb, wout_sb, id_sb, qT_sb, kT_sb,
                 v_sb, yT_r, fillers, pending, _it)
    while fillers:
        fillers.popleft()[1]()
    while pending:
        if "tail" in ABLATE:
            break
        ich, oTs, tfn = pending.pop(0)
        for _, u in tfn(ich, oTs):
            u()


def _compute(nc, psA, psB, ptp, sm, yp, ep, xT_sb, wq_sb, wk_sb, wv_sb,
             wout_sb, id_sb, qT_sb, kT_sb, v_sb, yT_r, fillers, pending, it):
    Exp = mybir.ActivationFunctionType.Exp
    Mul = mybir.AluOpType.mult
    Add = mybir.AluOpType.add
    F16 = mybir.dt.float16
    vb = it % 2

    # ---- polynomial exp on Pool+DVE for offloaded softmax tiles ----
    def dve_exp(st, out_ap):
        t = ep.tile([128, 1024], F16, tag="ea", name="t")
        nc.vector.tensor_scalar(out=t, in0=st, scalar1=EK1, scalar2=EK2,
                                op0=Mul, op1=Add)
        w1 = ep.tile([128, 1024], F16, tag="eb", name="w1")
        nc.vector.tensor_scalar_add(out=w1, in0=t, scalar1=ES1)
        w1s = ep.tile([128, 1024], F16, tag="ec", name="w1s")
        nc.vector.tensor_tensor(out=w1s, in0=w1, in1=w1, op=Mul)
        u1 = ep.tile([128, 1024], F16, tag="eb", name="u1")
        nc.vector.tensor_scalar_add(out=u1, in0=w1s, scalar1=EG1)
        w2 = ep.tile([128, 1024], F16, tag="ec", name="w2")
        nc.vector.tensor_scalar_add(out=w2, in0=t, scalar1=ES2)
        w2s = ep.tile([128, 1024], F16, tag="ed", name="w2s")
        nc.vector.tensor_tensor(out=w2s, in0=w2, in1=w2, op=Mul)
        u2 = ep.tile([128, 1024], F16, tag="ec", name="u2")
        nc.vector.tensor_scalar_add(out=u2, in0=w2s, scalar1=EG2)
        m1 = ep.tile([128, 1024], F16, tag="ed", name="m1")
        nc.vector.tensor_tensor(out=m1, in0=t, in1=u1, op=Mul)
        p = ep.tile([128, 1024], F16, tag="eb", name="p")
        nc.vector.tensor_tensor(out=p, in0=m1, in1=u2, op=Mul)
        e1 = ep.tile([128, 1024], F16, tag="ec", name="e1")
        nc.vector.tensor_tensor(out=e1, in0=p, in1=p, op=Mul)
        e2 = ep.tile([128, 1024], F16, tag="ed", name="e2")
        nc.vector.tensor_tensor(out=e2, in0=e1, in1=e1, op=Mul)
        nc.vector.tensor_tensor(out=out_ap, in0=e2, in1=e2, op=Mul)

    # ---- projections: q^T, k^T = W^T @ x^T. Chunk 0's first S group needs
    # kT chunks 0-1 and qT chunk 0 (emitted inline); the other 13 chunks
    # drain as fillers, deadline-interleaved with the previous iteration's
    # leftover AV units (kT chunk 2q..2q+1 gates this chunk's quarter q;
    # AV(7) blocks of 8 gate the P^T quarter-slot reuse). ----
    def proj_qk(w_sb, dst, nch):
        ns = bass.ts(nch, 512)
        pq = psA.tile([128, 1024], F32, tag="s", name="pq")
        for k in range(KT):
            nc.tensor.matmul(pq[:, 0:512], lhsT=w_sb[:, k, :],
                             rhs=xT_sb[:, k, ns],
                             start=(k == 0), stop=(k == KT - 1))
        nc.vector.tensor_copy(out=dst[:, ns], in_=pq[:, 0:512])

    if it == 0:
        # later iterations' kT0/kT1/qT0 recompute is hoisted into the
        # previous iteration's chunk-7 queue (prelude below)
        proj_qk(wk_sb, kT_sb, 0)
        proj_qk(wk_sb, kT_sb, 1)
        proj_qk(wq_sb, qT_sb, 0)
    punits = ([(1.3, lambda n=n: proj_qk(wk_sb, kT_sb, n))
               for n in range(2, NCH)]
              + [(1.3, lambda n=n: proj_qk(wq_sb, qT_sb, n))
                 for n in range(1, NCH)])
    left = list(fillers)
    fillers.clear()
    if len(left) >= 32:  # steady seam: [tail(6) ...][AV(7) x32]
        tl, av = left[:len(left) - 32], left[len(left) - 32:]
        fillers.extend([punits[0], punits[1]] + av[0:8]
                       + [punits[2], punits[3]] + av[8:16]
                       + [punits[4], punits[5]] + av[16:24]
                       + [punits[6]] + av[24:32] + tl + punits[7:])
    else:
        fillers.extend(left + punits)

    # ---- projection: v natural (x^T tiles stationary); emitted inside
    # i-chunk 0 per half, right before the AV that first consumes it ----
    def proj_v(jt):
        js = bass.ts(jt, 128)
        pv = psA.tile([128, 512], F32, tag="s", name="pv")
        for k in range(KT):
            nc.tensor.matmul(pv[:, 0:128], lhsT=xT_sb[:, k, js],
                             rhs=wv_sb[:, k, :],
                             start=(k == 0), stop=(k == KT - 1))
        nc.vector.tensor_copy(
            out=v_sb[:, vb, jt, :, 0:64],
            in_=pv[:, 0:128].rearrange("p (h d) -> p h d", h=2))

    # ---- tail: normalize by Z (PSUM col 64), transpose, out-project.
    # Returns a list of closures (filler units) so the PE work interleaves
    # with the next chunk's S/exp stream instead of blocking it. ----
    def tail_units(ich, oTs):
        ics = bass.ts(ich, 512)
        holder = {}

        def norm():
            oTn = holder["oTn"] = sm.tile([128, 4, 128], BF, tag="oTn",
                                          name="oTn")
            for hh in range(2):
                for isub in range(4):
                    rz = sm.tile([128, 1], F32, tag="rz", name="rz", bufs=4)
                    nc.vector.reciprocal(out=rz, in_=oTs[0][hh][:, isub, 64:65])
                    nc.vector.tensor_scalar_mul(
                        oTn[:, isub, hh * 64:hh * 64 + 64],
                        oTs[0][hh][:, isub, 0:64], rz)
            holder["osb"] = sm.tile([128, 512], BF, tag="osb", name="osb")

        def tr(isub):
            tps = psB.tile([128, 128], BF, tag="o", name="tps")
            nc.tensor.transpose(tps, holder["oTn"][:, isub, :], id_sb)
            nc.vector.tensor_copy(out=holder["osb"][:, bass.ts(isub, 128)],
                                  in_=tps)

        def proj(m):
            py = psB.tile([128, 512], F32, tag="o", name="py")
            nc.tensor.matmul(py, lhsT=wout_sb[:, m, :], rhs=holder["osb"],
                             start=True, stop=True)
            yb = yp.tile([128, 512], F32, tag="yb", name="yb")
            nc.vector.tensor_copy(out=yb, in_=py)
            nc.sync.dma_start(out=yT_r[m, :, ics], in_=yb)

        return ([(0.05, norm)]
                + [(0.35, lambda i=i: (tr(2 * i), tr(2 * i + 1)))
                   for i in range(2)]
                + [(0.55, lambda m=m: (proj(2 * m), proj(2 * m + 1)))
                   for m in range(KT // 2)])

    # ---- AV: P^T stationary (lhsT), rhs = [v | 1], Z lands in out col 64.
    # One quarter-pass per unit: the 4 i-sub accumulation chains of a head
    # run sequentially (PSUM allows only one open accumulation group per
    # bank), then DVE folds the quarter's partial o^T into an SBUF
    # accumulator. The P^T quarter-tile is fully consumed after its two
    # units, releasing its slot early for the next chunk's exp. ----
    def av_units(qts, oSs):
        def avq(q, hh):
            if "av" in ABLATE:
                return
            if q == 0 and hh == 0:
                oSs.append([sm.tile([128, 4, 65], F32, tag="oS", name="oS",
                                    bufs=4)
                            for _ in range(2)])
                oSs.append([psB.tile([128, 4, 68], F32, tag="o", name="oT")
                            for _ in range(2)])
            oS, oT = oSs[0][hh], oSs[1][hh]
            for isub in range(4):
                io = hh * 512 + isub * 128
                for jl in range(8):
                    nc.tensor.matmul(
                        oT[:, isub, 0:65],
                        lhsT=qts[q][:, jl, io:io + 128],
                        rhs=v_sb[:, vb, q * 8 + jl, hh, :],
                        start=(jl == 0), stop=(jl == 7))
            if q == 0:
                nc.vector.tensor_copy(out=oS, in_=oT[:, :, 0:65])
            else:
                nc.vector.tensor_add(oS, oS, oT[:, :, 0:65])
        return [(1.0, lambda q=q, hh=hh: avq(q, hh))
                for q in range(4) for hh in range(2)]

    # ---- attention per i-chunk; AV(c) and tail(c-1) drain as fillers
    # between chunk c+1's S/exp groups, budgeted so the PE work emitted per
    # group stays within that group's ScalarE time (values in ~us of PE) ----
    def drain(budget):
        while fillers and budget > 0:
            cost, fn = fillers.popleft()
            fn()
            budget -= cost

    for ich in range(NCH):
        ics = bass.ts(ich, 512)
        qts = []
        for q in range(4):
            ptb = ptp.tile([128, 8, 1024], BF, tag="pt", name="ptb")
            ptf = ptb.rearrange("p a b -> p (a b)")
            qts.append(ptb)
            g = 0
            for gi, csz in enumerate((2,) * 8):
                st = psA.tile([128, csz * 512], F32, tag="s", name="st")
                for s in range(csz):
                    jl, h = (g + s) // 2, (g + s) % 2
                    jt = q * 8 + jl
                    js = bass.ts(jt, 128)
                    if "s" in ABLATE:
                        continue
                    nc.tensor.matmul(st[:, bass.ts(s, 512)],
                                     lhsT=kT_sb[h * 64:h * 64 + 64, js],
                                     rhs=qT_sb[h * 64:h * 64 + 64, ics],
                                     start=True, stop=True,
                                     tile_position=(h * 64, 0))
                if "exp" not in ABLATE:
                    dst = ptf[:, g * 512:(g + csz) * 512]
                    if gi == 6 and q in (1, 3) and "dve" not in ABLATE:
                        dve_exp(st, dst)
                    else:
                        nc.scalar.activation(out=dst, in_=st,
                                             func=Exp, scale=SCALE)
                g += csz
                drain(0.55)
            if ich == NCH - 1 and q == 0:
                # prelude: next iteration's first projections, emitted after
                # this iteration's last reader of kT chunks 0-1 / qT chunk 0
                fillers.extendleft([
                    (1.3, lambda: proj_qk(wq_sb, qT_sb, 0)),
                    (1.3, lambda: proj_qk(wk_sb, kT_sb, 1)),
                    (1.3, lambda: proj_qk(wk_sb, kT_sb, 0))])
        if pending and "tail" not in ABLATE:
            pich, poTs, ptfn = pending.pop(0)
            fillers.extend(ptfn(pich, poTs))
        oSs = []
        avs = av_units(qts, oSs)
        if ich == 0:
            # refresh v (quarter q's 8 tiles) right before the first AV
            # quarter-pass that consumes it
            avs = [u for q in range(4)
                   for u in ([(0.42, lambda jt=jt: proj_v(jt))
                              for jt in range(q * 8, q * 8 + 8)]
                             + avs[2 * q:2 * q + 2])]
        fillers.extend(avs)
        pending.append((ich, oSs, tail_units))


_CACHE = {}


def _build():
    if "nc" not in _CACHE:
        nc = bacc.Bacc("TRN2", target_bir_lowering=False, debug=False,
                       num_devices=8)
        from contextlib import ExitStack
        with tile.TileContext(nc) as tc:
            with ExitStack() as ctx:
                _body(ctx, tc)
        nc.compile()
        _CACHE["nc"] = nc
    return _CACHE["nc"]


def make_in_maps(x, Wq, Wk, Wv, Wout):
    in_maps = []
    ident = np.eye(128, dtype=BF16)
    for core in range(8):
        b = core // 4
        sl = slice((core % 4) * 128, (core % 4) * 128 + 128)
        in_maps.append({
            "xT": x[b].T.astype(BF16),
            "wq": Wq[:, sl].astype(BF16),
            "wk": Wk[:, sl].astype(BF16),
            "wv": Wv[:, sl].astype(BF16),
            "wout": Wout[sl, :].astype(BF16),
            "ident": ident,
        })
    return in_maps


def kernel(x, Wq, Wk, Wv, Wout, b_out):
    x, Wq, Wk, Wv, Wout, b_out = (np.asarray(a) for a in
                                  (x, Wq, Wk, Wv, Wout, b_out))
    nc = _build()
    in_maps = make_in_maps(x, Wq, Wk, Wv, Wout)
    res = run_bass_kernel_spmd(nc, in_maps, core_ids=list(range(8)))
    y = np.zeros((B, N, QDIM), np.float32)
    for core in range(8):
        y[core // 4] += res.results[core]["yT"].T
    y += b_out.astype(np.float32)
    return y



# revision 6
# speedup vs baseline: 1.1646x; 1.1646x over previous
"""Multi-head self-attention on 8 Trainium2 NeuronCores.

Problem: x[2, 4096, 768], Wq/Wk/Wv[768, 512], Wout[512, 768], b_out[768]
  q,k,v = heads(x@W*); S = qk^T/8; P = softmax(S); out = (P v) @ Wout + b_out
Sharding: 16 (batch, head) pairs -> 8 cores, 2 heads each (core i: batch i//4,
  heads 2*(i%4), 2*(i%4)+1). Each core holds its weight slices and computes a
  partial y^T[768, 4096]; host sums the 4 per-batch partials and adds b_out.

Device dataflow (all matmuls bf16, fp32 PSUM accumulation):
  x^T (transposed+cast on host)  ->  q^T,k^T [128, 4096]  (W stationary)
  v natural [4096, 128] via x^T-stationary matmuls, stored with a ones
    column per head (v_ext[j, h, 0:65], col 64 = 1.0)
  S^T[j,i] both heads per j-tile via row-tiled (tile_position (0,0)/(64,0))
    K=64 matmul pairs into [128, 1536] PSUM chunks (3 banks, double-buffered)
  P^T = exp(S^T/8) on ScalarE in 1536-wide ops (no max subtraction; scores
    are O(5) so fp32 exp is safe) -- ScalarE is the bottleneck engine
  AV with P^T STATIONARY (lhsT): per (head, i-sub 128) chain over 32 j-tiles,
    rhs = v_ext[j, h, 0:65] (N=65) -> o^T[i, 0:64] and Z[i] in col 64 of the
    same PSUM accumulation, so the softmax denominator is free (this removes
    the DVE zacc accumulation and the z-broadcast matmuls of the previous
    version, and halves the AV charge on PE: 65 output rows vs 2x512).
  normalize: rz = 1/Z (DVE reciprocal), o^T scaled per-partition into bf16
    oTn (DVE tensor_scalar_mul); PE-transpose oTn -> o[d, i] (identity rhs)
  out-proj: single K=128 matmul per 128-row slice of Wout; tail of chunk i
    is emitted after chunk i+1's S/exp so it never gates ScalarE.
  Bulk PSUM->SBUF copies (q^T/k^T, v, y) run on the otherwise idle GpSimd
    (Pool) engine.
"""
import os
import numpy as np
import ml_dtypes

ABLATE = set(os.environ.get("KABLATE", "").split(","))
KITER = int(os.environ.get("KITER", "1"))
# softmax groups (of 8 per i-chunk quarter) whose exp runs on DVE not ScalarE
DVE_GI = frozenset(
    int(x) for x in os.environ.get("KDVEGI", "2,6").split(",") if x != "")

import concourse.bass as bass
import concourse.mybir as mybir
import concourse.tile as tile
from concourse import bacc
from concourse.bass_utils import run_bass_kernel_spmd

BF16 = ml_dtypes.bfloat16
F32 = mybir.dt.float32
BF = mybir.dt.bfloat16

B, N, QDIM = 2, 4096, 768
H, D = 8, 64
KT = QDIM // 128          # 6 contraction tiles
NCH = N // 512            # 8 i-chunks
NJT = N // 128            # 32 j-tiles
SCALE = D ** -0.5         # 1/8

# DVE 1-op "Schraudolph" exp for offloaded softmax tiles: bf16's bit layout
# is sign(1)|exp(8)|man(7), so int16 i = round(a*z + b) with a = 128/ln2,
# b = 127*128 + sigma reinterpreted as bf16 gives 2^(z/ln2) with a piecewise-
# linear mantissa (rel err ~1.8% rms, ~4% max; sigma = -7.5 centers the mean,
# +0.5 compensates truncate-on-convert). The fp32 PSUM read and int16 SBUF
# write (bitcast over the bf16 P tile) happen in one tensor_scalar pass.
import math
SCHR_A = 128 / math.log(2) / 8     # x SCALE folded in
SCHR_B = 127 * 128 - 7.5 + 0.5


def _body(ctx, tc):
    nc = tc.nc

    xT = nc.dram_tensor("xT", [QDIM, N], BF, kind="ExternalInput").ap()
    wq = nc.dram_tensor("wq", [QDIM, 128], BF, kind="ExternalInput").ap()
    wk = nc.dram_tensor("wk", [QDIM, 128], BF, kind="ExternalInput").ap()
    wv = nc.dram_tensor("wv", [QDIM, 128], BF, kind="ExternalInput").ap()
    wout = nc.dram_tensor("wout", [128, QDIM], BF, kind="ExternalInput").ap()
    ident = nc.dram_tensor("ident", [128, 128], BF, kind="ExternalInput").ap()
    yT = nc.dram_tensor("yT", [QDIM, N], F32, kind="ExternalOutput").ap()

    xT_r = xT.rearrange("(k p) n -> p k n", p=128)
    wq_r = wq.rearrange("(k p) m -> p k m", p=128)
    wk_r = wk.rearrange("(k p) m -> p k m", p=128)
    wv_r = wv.rearrange("(k p) m -> p k m", p=128)
    wout_r = wout.rearrange("p (k f) -> p k f", f=128)
    yT_r = yT.rearrange("(m p) n -> m p n", p=128)

    # ---- static SBUF ----
    singles = ctx.enter_context(tc.tile_pool(name="singles", bufs=1))
    xT_sb = singles.tile([128, KT, N], BF, name="xT_sb", tag="xT_sb")
    wq_sb = singles.tile([128, KT, 128], BF, name="wq_sb", tag="wq_sb")
    wk_sb = singles.tile([128, KT, 128], BF, name="wk_sb", tag="wk_sb")
    wv_sb = singles.tile([128, KT, 128], BF, name="wv_sb", tag="wv_sb")
    wout_sb = singles.tile([128, KT, 128], BF, name="wout_sb", tag="wout_sb")
    id_sb = singles.tile([128, 128], BF, name="id_sb", tag="id_sb")
    qT_sb = singles.tile([128, N], BF, name="qT_sb", tag="qT_sb")
    kT_sb = singles.tile([128, N], BF, name="kT_sb", tag="kT_sb")
    # v per j-tile and head, with a trailing ones column: [j, jt, h, 0:64]=v,
    # [j, jt, h, 64]=1.0 (the AV rhs [j, 65] then accumulates Z in out col 64).
    # Double-buffered across KITER iterations so iteration n+1's v projection
    # does not serialize behind iteration n's AV reads.
    v_sb = singles.tile([128, 2, NJT, 2, 65], BF, name="v_sb", tag="v_sb")

    for k in range(KT):
        for q4 in range(4):
            qs = bass.ts(q4, N // 4)
            nc.sync.dma_start(out=xT_sb[:, k, qs], in_=xT_r[:, k, qs])
    nc.sync.dma_start(out=wq_sb, in_=wq_r)
    nc.sync.dma_start(out=wk_sb, in_=wk_r)
    nc.sync.dma_start(out=wv_sb, in_=wv_r)
    nc.sync.dma_start(out=wout_sb, in_=wout_r)
    nc.sync.dma_start(out=id_sb, in_=ident)
    nc.vector.memset(v_sb[:, :, :, :, 64], 1.0)

    psA = ctx.enter_context(tc.tile_pool(name="psA", bufs=3, space="PSUM"))
    psB = ctx.enter_context(tc.tile_pool(name="psB", bufs=2, space="PSUM"))
    ptp = ctx.enter_context(tc.tile_pool(name="ptp", bufs=6))
    sm = ctx.enter_context(tc.tile_pool(name="sm", bufs=2))
    yp = ctx.enter_context(tc.tile_pool(name="yp", bufs=2))

    from collections import deque
    fillers = deque()
    pending = []
    for _it in range(KITER):
        _compute(nc, psA, psB, ptp, sm, yp,
                 xT_sb, wq_sb, wk_sb, wv_sb, wout_sb, id_sb, qT_sb, kT_sb,
                 v_sb, yT_r, fillers, pending, _it)
    while fillers:
        fillers.popleft()[1]()
    while pending:
        if "tail" in ABLATE:
            break
        ich, oTs, tfn = pending.pop(0)
        for _, u in tfn(ich, oTs):
            u()


def _compute(nc, psA, psB, ptp, sm, yp, xT_sb, wq_sb, wk_sb, wv_sb,
             wout_sb, id_sb, qT_sb, kT_sb, v_sb, yT_r, fillers, pending, it):
    Exp = mybir.ActivationFunctionType.Exp
    Mul = mybir.AluOpType.mult
    Add = mybir.AluOpType.add
    I16 = mybir.dt.int16
    vb = it % 2

    # ---- 1-op Schraudolph exp on DVE for offloaded softmax tiles ----
    def dve_exp(st, out_ap):
        nc.vector.tensor_scalar(out=out_ap.bitcast(I16), in0=st,
                                scalar1=SCHR_A, scalar2=SCHR_B,
                                op0=Mul, op1=Add)

    # ---- projections: q^T, k^T = W^T @ x^T. Chunk 0's first S group needs
    # kT chunks 0-1 and qT chunk 0 (emitted inline); the other 13 chunks
    # drain as fillers, deadline-interleaved with the previous iteration's
    # leftover AV units (kT chunk 2q..2q+1 gates this chunk's quarter q;
    # AV(7) blocks of 8 gate the P^T quarter-slot reuse). ----
    def proj_qk(w_sb, dst, nch):
        ns = bass.ts(nch, 512)
        pq = psA.tile([128, 1024], F32, tag="s", name="pq")
        for k in range(KT):
            nc.tensor.matmul(pq[:, 0:512], lhsT=w_sb[:, k, :],
                             rhs=xT_sb[:, k, ns],
                             start=(k == 0), stop=(k == KT - 1))
        nc.vector.tensor_copy(out=dst[:, ns], in_=pq[:, 0:512])

    if it == 0:
        # later iterations' kT0/kT1/qT0 recompute is hoisted into the
        # previous iteration's chunk-7 queue (prelude below)
        proj_qk(wk_sb, kT_sb, 0)
        proj_qk(wk_sb, kT_sb, 1)
        proj_qk(wq_sb, qT_sb, 0)
    punits = ([(1.3, lambda n=n: proj_qk(wk_sb, kT_sb, n))
               for n in range(2, NCH)]
              + [(1.3, lambda n=n: proj_qk(wq_sb, qT_sb, n))
                 for n in range(1, NCH)])
    left = list(fillers)
    fillers.clear()
    if len(left) >= 32:  # steady seam: [tail(6) ...][AV(7) x32]
        tl, av = left[:len(left) - 32], left[len(left) - 32:]
        fillers.extend([punits[0], punits[1]] + av[0:8]
                       + [punits[2], punits[3]] + av[8:16]
                       + [punits[4], punits[5]] + av[16:24]
                       + [punits[6]] + av[24:32] + tl + punits[7:])
    else:
        fillers.extend(left + punits)

    # ---- projection: v natural (x^T tiles stationary); emitted inside
    # i-chunk 0 per half, right before the AV that first consumes it ----
    def proj_v(jt):
        js = bass.ts(jt, 128)
        pv = psA.tile([128, 512], F32, tag="s", name="pv")
        for k in range(KT):
            nc.tensor.matmul(pv[:, 0:128], lhsT=xT_sb[:, k, js],
                             rhs=wv_sb[:, k, :],
                             start=(k == 0), stop=(k == KT - 1))
        nc.vector.tensor_copy(
            out=v_sb[:, vb, jt, :, 0:64],
            in_=pv[:, 0:128].rearrange("p (h d) -> p h d", h=2))

    # ---- tail: normalize by Z (PSUM col 64), transpose, out-project.
    # Returns a list of closures (filler units) so the PE work interleaves
    # with the next chunk's S/exp stream instead of blocking it. ----
    def tail_units(ich, oTs):
        ics = bass.ts(ich, 512)
        holder = {}

        def norm():
            oTn = holder["oTn"] = sm.tile([128, 4, 128], BF, tag="oTn",
                                          name="oTn")
            for hh in range(2):
                for isub in range(4):
                    rz = sm.tile([128, 1], F32, tag="rz", name="rz", bufs=4)
                    nc.vector.reciprocal(out=rz, in_=oTs[0][hh][:, isub, 64:65])
                    nc.vector.tensor_scalar_mul(
                        oTn[:, isub, hh * 64:hh * 64 + 64],
                        oTs[0][hh][:, isub, 0:64], rz)
            holder["osb"] = sm.tile([128, 512], BF, tag="osb", name="osb")

        def tr(isub):
            tps = psB.tile([128, 128], BF, tag="o", name="tps")
            nc.tensor.transpose(tps, holder["oTn"][:, isub, :], id_sb)
            nc.vector.tensor_copy(out=holder["osb"][:, bass.ts(isub, 128)],
                                  in_=tps)

        def proj(m):
            py = psB.tile([128, 512], F32, tag="o", name="py")
            nc.tensor.matmul(py, lhsT=wout_sb[:, m, :], rhs=holder["osb"],
                             start=True, stop=True)
            yb = yp.tile([128, 512], F32, tag="yb", name="yb")
            nc.vector.tensor_copy(out=yb, in_=py)
            nc.sync.dma_start(out=yT_r[m, :, ics], in_=yb)

        return ([(0.05, norm)]
                + [(0.35, lambda i=i: (tr(2 * i), tr(2 * i + 1)))
                   for i in range(2)]
                + [(0.55, lambda m=m: (proj(2 * m), proj(2 * m + 1)))
                   for m in range(KT // 2)])

    # ---- AV: P^T stationary (lhsT), rhs = [v | 1], Z lands in out col 64.
    # One quarter-pass per unit: the 4 i-sub accumulation chains of a head
    # run sequentially (PSUM allows only one open accumulation group per
    # bank), then DVE folds the quarter's partial o^T into an SBUF
    # accumulator. The P^T quarter-tile is fully consumed after its two
    # units, releasing its slot early for the next chunk's exp. ----
    def av_units(qts, oSs):
        def avq(q, hh):
            if "av" in ABLATE:
                return
            if q == 0 and hh == 0:
                oSs.append([sm.tile([128, 4, 65], F32, tag="oS", name="oS",
                                    bufs=4)
                            for _ in range(2)])
                oSs.append([psB.tile([128, 4, 68], F32, tag="o", name="oT")
                            for _ in range(2)])
            oS, oT = oSs[0][hh], oSs[1][hh]
            for isub in range(4):
                io = hh * 512 + isub * 128
                for jl in range(8):
                    nc.tensor.matmul(
                        oT[:, isub, 0:65],
                        lhsT=qts[q][:, jl, io:io + 128],
                        rhs=v_sb[:, vb, q * 8 + jl, hh, :],
                        start=(jl == 0), stop=(jl == 7))
            if q == 0:
                nc.vector.tensor_copy(out=oS, in_=oT[:, :, 0:65])
            else:
                nc.vector.tensor_add(oS, oS, oT[:, :, 0:65])
        return [(1.0, lambda q=q, hh=hh: avq(q, hh))
                for q in range(4) for hh in range(2)]

    # ---- attention per i-chunk; AV(c) and tail(c-1) drain as fillers
    # between chunk c+1's S/exp groups, budgeted so the PE work emitted per
    # group stays within that group's ScalarE time (values in ~us of PE) ----
    def drain(budget):
        while fillers and budget > 0:
            cost, fn = fillers.popleft()
            fn()
            budget -= cost

    for ich in range(NCH):
        ics = bass.ts(ich, 512)
        qts = []
        for q in range(4):
            ptb = ptp.tile([128, 8, 1024], BF, tag="pt", name="ptb")
            ptf = ptb.rearrange("p a b -> p (a b)")
            qts.append(ptb)
            g = 0
            for gi, csz in enumerate((2,) * 8):
                st = psA.tile([128, csz * 512], F32, tag="s", name="st")
                for s in range(csz):
                    jl, h = (g + s) // 2, (g + s) % 2
                    jt = q * 8 + jl
                    js = bass.ts(jt, 128)
                    if "s" in ABLATE:
                        continue
                    nc.tensor.matmul(st[:, bass.ts(s, 512)],
                                     lhsT=kT_sb[h * 64:h * 64 + 64, js],
                                     rhs=qT_sb[h * 64:h * 64 + 64, ics],
                                     start=True, stop=True,
                                     tile_position=(h * 64, 0))
                if "exp" not in ABLATE:
                    dst = ptf[:, g * 512:(g + csz) * 512]
                    if gi in DVE_GI and "dve" not in ABLATE:
                        dve_exp(st, dst)
                    else:
                        nc.scalar.activation(out=dst, in_=st,
                                             func=Exp, scale=SCALE)
                g += csz
                drain(0.55)
            if ich == NCH - 1 and q == 0:
                # prelude: next iteration's first projections, emitted after
                # this iteration's last reader of kT chunks 0-1 / qT chunk 0
                fillers.extendleft([
                    (1.3, lambda: proj_qk(wq_sb, qT_sb, 0)),
                    (1.3, lambda: proj_qk(wk_sb, kT_sb, 1)),
                    (1.3, lambda: proj_qk(wk_sb, kT_sb, 0))])
        if pending and "tail" not in ABLATE:
            pich, poTs, ptfn = pending.pop(0)
            fillers.extend(ptfn(pich, poTs))
        oSs = []
        avs = av_units(qts, oSs)
        if ich == 0:
            # refresh v (quarter q's 8 tiles) right before the first AV
            # quarter-pass that consumes it
            avs = [u for q in range(4)
                   for u in ([(0.42, lambda jt=jt: proj_v(jt))
                              for jt in range(q * 8, q * 8 + 8)]
                             + avs[2 * q:2 * q + 2])]
        fillers.extend(avs)
        pending.append((ich, oSs, tail_units))


_CACHE = {}


def _build():
    if "nc" not in _CACHE:
        nc = bacc.Bacc("TRN2", target_bir_lowering=False, debug=False,
                       num_devices=8)
        from contextlib import ExitStack
        with tile.TileContext(nc) as tc:
            with ExitStack() as ctx:
                _body(ctx, tc)
        nc.compile()
        _CACHE["nc"] = nc
    return _CACHE["nc"]


def make_in_maps(x, Wq, Wk, Wv, Wout):
    in_maps = []
    ident = np.eye(128, dtype=BF16)
    for core in range(8):
        b = core // 4
        sl = slice((core % 4) * 128, (core % 4) * 128 + 128)
        in_maps.append({
            "xT": x[b].T.astype(BF16),
            "wq": Wq[:, sl].astype(BF16),
            "wk": Wk[:, sl].astype(BF16),
            "wv": Wv[:, sl].astype(BF16),
            "wout": Wout[sl, :].astype(BF16),
            "ident": ident,
        })
    return in_maps


def kernel(x, Wq, Wk, Wv, Wout, b_out):
    x, Wq, Wk, Wv, Wout, b_out = (np.asarray(a) for a in
                                  (x, Wq, Wk, Wv, Wout, b_out))
    nc = _build()
    in_maps = make_in_maps(x, Wq, Wk, Wv, Wout)
    res = run_bass_kernel_spmd(nc, in_maps, core_ids=list(range(8)))
    y = np.zeros((B, N, QDIM), np.float32)
    for core in range(8):
        y[core // 4] += res.results[core]["yT"].T
    y += b_out.astype(np.float32)
    return y

